# revision 49
# baseline (speedup 1.0000x reference)
"""Multi-head attention (B=4, S=2048, D=1024, H=16) on 8 Trainium2 NeuronCores.

Sharding: core c = (batch b = c//2, head-group hg = c%2). Each core computes
heads hg*8..hg*8+7 for batch b over the full sequence, producing a partial
output o_c[s, :] = ctx_c @ Wo[:, hg-dims].T (+ bo on hg==0 cores). The host
sums the two partial outputs per batch. This is an exact decomposition: each
core does 1/8 of the total FLOPs with no cross-core communication.

Per-core dataflow (all matmul inputs bf16, accumulation fp32):
  phase 1: KT/QT = W @ z.T feature-major (lhsT = W.T tiles, rhs = z.T tiles);
           V token-major (lhsT = z.T tiles, rhs = Wv.T chunk).
  phase 2: per head pair, per 1024-query pass: the two heads' scoresT[k, q]
           matmuls are emitted adjacently so their K=64 weight tiles occupy
           the top/bottom halves of the PE array (row tiling via
           tile_position auto-derivation) and run concurrently; exp on
           ScalarE (scale=1/8 fused, max-subtraction dropped -- scores are
           bounded ~N(0,1/3)); the two heads' probs @ V matmuls are col-tiled
           (M=64 each) into one shared [128, q] PSUM tile, also concurrent.
           Softmax denominators: DVE pairwise-adds the 16 probs tiles per
           head (bf16 tree), then a single ones[128,64]-weights matmul
           reduces the 128 partial sums and broadcasts the result to 64
           partitions in one shot; reciprocal + normalize + bv on VectorE.
           bk is dropped (softmax shift invariance); bv is added
           post-normalization (exact identity since sum_k p[k] = denom).
  phase 3: o[s, j] = ctxT.T @ Wo.T partial contraction (+ bo via DVE add).

The ScalarE exp stream (256 x [128,1024] activations, ~270us) is the
intended critical path; PE work (~230us) hides underneath it.
"""

from contextlib import ExitStack

import ml_dtypes
import numpy as np

import concourse.bass as bass
import concourse.tile as tile
from concourse import bacc, mybir
from concourse.bass_utils import run_bass_kernel_spmd

BF16 = mybir.dt.bfloat16
F32 = mybir.dt.float32
NPBF16 = ml_dtypes.bfloat16

B, S, D, H, DK = 4, 2048, 1024, 16, 64
N_CORES = 8
HG = H // 2  # heads per core
NPAIR = HG // 2  # head pairs per core
ND = D // 128  # contraction d-tiles
NT = S // 128  # token tiles
NQP = 2  # query passes of 1024
QW = S // NQP  # query window
DH = HG * DK  # 512: output dims per core
SCALE = 1.0 / np.sqrt(DK)
EXP = mybir.ActivationFunctionType.Exp


def _emit(tc, tin, tout):
    nc = tc.nc
    with ExitStack() as ctx:
        SP = ctx.enter_context(tc.tile_pool(name="static", bufs=1))
        PS = ctx.enter_context(tc.tile_pool(name="psum", bufs=2, space="PSUM"))
        KTP = ctx.enter_context(tc.tile_pool(name="ktp", bufs=2))
        QTP = ctx.enter_context(tc.tile_pool(name="qtp", bufs=2))
        WKP = ctx.enter_context(tc.tile_pool(name="wkp", bufs=2))
        WQP = ctx.enter_context(tc.tile_pool(name="wqp", bufs=2))
        PTP = ctx.enter_context(tc.tile_pool(name="ptp", bufs=16))
        TRP = ctx.enter_context(tc.tile_pool(name="trp", bufs=2))
        RCP = ctx.enter_context(tc.tile_pool(name="rcp", bufs=2))
        OSP = ctx.enter_context(tc.tile_pool(name="osp", bufs=3))

        # ---- constants ----
        bq_all = SP.tile([128, NPAIR], F32, tag="bq_all")
        bv_all = SP.tile([128, NPAIR], F32, tag="bv_all")
        ones_red = SP.tile([128, DK], BF16, tag="ones_red")
        nc.vector.memset(ones_red[:], 1.0)
        zexp = SP.tile([128, 1], F32, tag="zexp")
        nc.vector.memset(zexp[:], 0.0)
        # tiny exp to pull the ACT table load off the critical path: it
        # issues during the initial DMA wait
        warm = SP.tile([1, 1], BF16, tag="warm")
        nc.scalar.activation(warm[:], zexp[0:1, :], EXP, bias=zexp[0:1, :])
        # dummy matmuls during the initial DMA wait: ~4us of PE activity
        # flips the HAM clock gate to 8/8 so the first projections run at
        # 2.4 GHz instead of 1.2
        wmrhs = SP.tile([128, 512], BF16, tag="wmrhs")
        nc.vector.memset(wmrhs[:], 0.0)
        wmps = PS.tile([64, 512], F32, tag="chunk", bufs=2, name="wmps")
        for _ in range(6):
            nc.tensor.matmul(
                wmps[:], lhsT=ones_red[:], rhs=wmrhs[:], start=True, stop=True
            )

        # ---- static loads ----
        # z.T d-tiles loaded in column-quarters, quarter-major, spread over
        # DGE streams.  Each (quarter, d) is its OWN tile: a single [128, S]
        # tile per d would chain its four quarter-DMAs on write-after-write
        # semaphores, serializing the startup loads.
        dmae = [nc.sync, nc.gpsimd]
        zq = [
            [
                SP.tile([128, 512], BF16, tag=f"zq{q_}d{d}", name=f"zq{q_}d{d}")
                for d in range(ND)
            ]
            for q_ in range(4)
        ]

        def zts(d, sl):
            # slice of z.T d-tile: sl must stay within one 512-col quarter
            q_, off = sl.start // 512, sl.start % 512
            assert sl.stop <= (q_ + 1) * 512
            return zq[q_][d][:, off : off + (sl.stop - sl.start)]

        wvs = [
            SP.tile([128, DH], BF16, tag=f"wv{d}", name=f"wv{d}") for d in range(ND)
        ]

        dmae4 = [nc.sync, nc.gpsimd, nc.scalar]

        def load_z_quarter(quarter, wide=False):
            csl = slice(quarter * 512, (quarter + 1) * 512)
            eng = dmae4 if wide else dmae
            for d in range(ND):
                eng[d % len(eng)].dma_start(
                    zq[quarter][d][:], tin["ztc"][d * 128 : (d + 1) * 128, csl]
                )

        def load_wv():
            for d in range(ND):
                dmae[d % 2].dma_start(
                    wvs[d][:], tin["wvTc"][d * 128 : (d + 1) * 128, :]
                )

        # V tiles: [128 tokens, 8 heads x 64 dims]
        vsb = [
            SP.tile([128, DH], BF16, tag=f"vsb{t}", name=f"vsb{t}")
            for t in range(NT)
        ]

        ctxu = []
        for lj in range(NPAIR):
            cu = SP.tile([128, S], BF16, tag=f"ctxu{lj}", name=f"ctxu{lj}")
            ctxu.append(cu)

        def emit_vproj(t):
            ps = PS.tile([128, DH], F32, tag="chunk", bufs=2, name=f"psv{t}")
            for d in range(ND):
                nc.tensor.matmul(
                    ps[:],
                    lhsT=zts(d, slice(t * 128, (t + 1) * 128)),
                    rhs=wvs[d][:],
                    start=(d == 0),
                    stop=(d == ND - 1),
                )
            nc.vector.tensor_copy(vsb[t][:], ps[:])

        def emit_proj_dmas(lj):
            # all 8 d-tiles of a [1024, 128] weight column-block in one DMA:
            # out[p, d, j] <- w[d*128+p, lj*128+j]
            jsl = slice(lj * 128, (lj + 1) * 128)
            wkj = WKP.tile([128, ND * 128], BF16, tag="wk", name=f"wk_{lj}")
            nc.sync.dma_start(
                wkj.rearrange("p (d j) -> p d j", j=128),
                tin["wkTc"][:, jsl].rearrange("(d p) j -> p d j", p=128),
            )
            wqj = WQP.tile([128, ND * 128], BF16, tag="wq", name=f"wq_{lj}")
            nc.gpsimd.dma_start(
                wqj.rearrange("p (d j) -> p d j", j=128),
                tin["wqTc"][:, jsl].rearrange("(d p) j -> p d j", p=128),
            )
            kt = KTP.tile([128, S], BF16, tag="kt", name=f"kt{lj}")
            qt = QTP.tile([128, S], BF16, tag="qt", name=f"qt{lj}")
            return (lj, wkj, wqj, kt, qt)

        chunk_pend = {}

        def emit_proj_half(pst, i, half):
            """Half of a K/Q projection chunk: 4 of the 8 contraction
            matmuls.  Splitting a chunk over two k-iterations keeps the
            per-iteration PE load under the exp-stream pace."""
            lj, wkj, wqj, kt, qt = pst
            tcx = i % (S // 512)
            sl = slice(tcx * 512, (tcx + 1) * 512)
            w = wkj if i < S // 512 else wqj
            key = (lj, i)
            if half == 0:
                chunk_pend[key] = PS.tile(
                    [128, 512], F32, tag="chunk", bufs=2, name=f"psh{lj}_{i}"
                )
            ps = chunk_pend[key]
            for d in range(4 * half, 4 * half + 4):
                nc.tensor.matmul(
                    ps[:],
                    lhsT=w[:, d * 128 : (d + 1) * 128],
                    rhs=zts(d, sl),
                    start=(d == 0),
                    stop=(d == ND - 1),
                )
            if half == 1:
                del chunk_pend[key]
                if i < S // 512:
                    nc.vector.tensor_copy(kt[:, sl], ps[:])
                else:
                    nc.vector.tensor_scalar_add(
                        qt[:, sl], ps[:], bq_all[:, lj : lj + 1]
                    )

        def emit_proj_chunk(pst, i):
            """One K- or Q-projection psum group (8 matmuls + evac)."""
            lj, wkj, wqj, kt, qt = pst
            tcx = i % (S // 512)
            sl = slice(tcx * 512, (tcx + 1) * 512)
            if i < S // 512:
                psk = PS.tile([128, 512], F32, tag="chunk", bufs=2, name=f"psk{lj}_{tcx}")
                for d in range(ND):
                    nc.tensor.matmul(
                        psk[:],
                        lhsT=wkj[:, d * 128 : (d + 1) * 128],
                        rhs=zts(d, sl),
                        start=(d == 0),
                        stop=(d == ND - 1),
                    )
                nc.vector.tensor_copy(kt[:, sl], psk[:])
            else:
                psq = PS.tile([128, 512], F32, tag="chunk", bufs=2, name=f"psq{lj}_{tcx}")
                for d in range(ND):
                    nc.tensor.matmul(
                        psq[:],
                        lhsT=wqj[:, d * 128 : (d + 1) * 128],
                        rhs=zts(d, sl),
                        start=(d == 0),
                        stop=(d == ND - 1),
                    )
                nc.vector.tensor_scalar_add(qt[:, sl], psq[:], bq_all[:, lj : lj + 1])

        # ---- lead-in: minimal prefix to get the first exps going fast ----
        # pair-0 weight DMAs (tiny) and z-quarter-0 gate the first K/Q
        # chunks; Wv and the remaining z quarters stream in behind them
        proj0 = emit_proj_dmas(0)
        # quarters 0/1 gate the first K/Q projections (and thus the first
        # scores/exp): spray them over four DGE queues; wv is only needed
        # once the PV stream starts
        load_z_quarter(0, wide=True)
        load_z_quarter(1, wide=True)
        nc.sync.dma_start(bq_all[:], tin["bqc"][:, :])
        nc.sync.dma_start(bv_all[:], tin["bvc"][:, :])
        load_wv()
        load_z_quarter(2)
        load_z_quarter(3)
        emit_proj_chunk(proj0, 0)  # K tokens 0..511
        emit_proj_chunk(proj0, 4)  # Q tokens 0..511
        emit_proj_chunk(proj0, 5)  # Q tokens 512..1023
        # remaining chunks are spread through block (0,0): K-chunk c is
        # first needed by scores k-tile 4c, Q chunks 2/3 only by block (0,1)
        proj0_rest = {2: 1, 4: 6, 6: 2, 8: 7, 10: 3}

        def emit_phase3(st, tail=False):
            # bo is added host-side during the cross-core reduction
            ost = OSP.tile([128, D], F32, tag="ost", name=f"ost{st}")
            ssl = slice(st * 128, (st + 1) * 128)
            for jc in range(2):
                jsl = slice(jc * 512, (jc + 1) * 512)
                # in the tail both psum rings are draining, so alternate
                # tags for a 4-deep rotation that keeps the PE pipelined
                tag = ("chunk", "ps")[(st + jc) % 2] if tail else "chunk"
                ps = PS.tile([128, 512], F32, tag=tag, bufs=2, name=f"pso{st}_{jc}")
                for l in range(NPAIR):
                    nc.tensor.matmul(
                        ps[:], lhsT=ctxu[l][:, ssl], rhs=wos[l][:, jsl],
                        start=(l == 0), stop=(l == NPAIR - 1),
                    )
                # tail evacuations go on ScalarE (idle after the exp stream
                # ends) so the phase-3 chain does not serialize on VectorE
                if tail:
                    nc.scalar.copy(ost[:, jsl], ps[:])
                else:
                    nc.vector.tensor_copy(ost[:, jsl], ps[:])
            nc.sync.dma_start(tout["o"][ssl, :], ost[:])

        # ---- attention blocks ----
        wos = []
        kt_cur, qt_cur = proj0[3], proj0[4]
        kt_next = qt_next = None
        proj_cur = proj0
        blocks = [(lj, qp) for lj in range(NPAIR) for qp in range(NQP)]
        last = len(blocks) - 1
        for bi, (lj, qp) in enumerate(blocks):
            if bi == 5:
                # phase-3 weights, loaded off the startup critical path
                for pl in range(NPAIR):
                    wo_ = SP.tile([128, D], BF16, tag=f"wo{pl}", name=f"wo{pl}")
                    nc.sync.dma_start(
                        wo_[:], tin["woTc"][pl * 128 : (pl + 1) * 128, :]
                    )
                    wos.append(wo_)
            q0 = qp * QW
            h0 = 2 * lj
            # shared PV accumulator: head0 dims on partitions 0:64, head1 on
            # 64:128 (col-tiled concurrent PV matmuls)
            ctx01 = PS.tile([128, QW], F32, tag="ctx", bufs=1, name=f"ctx_{lj}_{qp}")
            # denominator accumulation per head: pair-add adjacent probs
            # tiles, then fold each pair-sum into a running total.  The
            # end-of-block serial chain is only two adds (pair + fold).
            half = [None, None]  # pending unpaired probs tile
            rsum = [None, None]  # running sum of pair-adds
            nl1 = [0, 0]  # pair-adds emitted so far
            prev = []  # deferred PV work: (kk, p0, p1)

            def tree_push(h, t):
                if half[h] is None:
                    half[h] = t
                    return
                l1 = TRP.tile([128, QW], BF16, tag=f"l0h{h}", name=f"l0h{h}_{bi}")
                nc.vector.tensor_add(l1[:], half[h][:], t[:])
                nl1[h] += 1
                half[h] = None
                if rsum[h] is None:
                    rsum[h] = l1
                else:
                    rs = TRP.tile([128, QW], BF16, tag=f"rsh{h}", name=f"rsh{h}_{bi}")
                    nc.vector.tensor_add(rs[:], rsum[h][:], l1[:])
                    rsum[h] = rs

            def emit_pv(kk, pp0, pp1):
                v0 = vsb[kk][:, h0 * DK : (h0 + 1) * DK]
                v1 = vsb[kk][:, (h0 + 1) * DK : (h0 + 2) * DK]
                for qc in range(2):
                    psl = slice(qc * 512, (qc + 1) * 512)
                    nc.tensor.matmul(
                        ctx01[0:64, psl], lhsT=v0, rhs=pp0[:, psl],
                        start=(kk == 0), stop=(kk == NT - 1),
                    )
                    nc.tensor.matmul(
                        ctx01[64:128, psl], lhsT=v1, rhs=pp1[:, psl],
                        start=(kk == 0), stop=(kk == NT - 1),
                    )

            for k in range(NT):
                ksl = slice(k * 128, (k + 1) * 128)
                # scores: the two heads' K=64 matmuls emitted adjacently so
                # they row-tile into the top/bottom array halves and run
                # concurrently (tile_position auto-derived from partitions)
                s0 = PS.tile([128, QW], F32, tag="ps", name=f"s0_{bi}_{k}")
                s1 = PS.tile([128, QW], F32, tag="ps", name=f"s1_{bi}_{k}")
                # high priority: the exp stream is the critical path, so its
                # producers must preempt PV/projection backlog on the PE
                with tc.high_priority():
                    for qc in range(2):
                        psl = slice(qc * 512, (qc + 1) * 512)
                        qsl = slice(q0 + qc * 512, q0 + (qc + 1) * 512)
                        nc.tensor.matmul(
                            s0[:, psl],
                            lhsT=kt_cur[0:64, ksl],
                            rhs=qt_cur[0:64, qsl],
                            start=True, stop=True,
                        )
                        nc.tensor.matmul(
                            s1[:, psl],
                            lhsT=kt_cur[64:128, ksl],
                            rhs=qt_cur[64:128, qsl],
                            start=True, stop=True,
                        )
                p0 = PTP.tile([128, QW], BF16, tag="pt", name=f"p0_{bi}_{k}")
                nc.scalar.activation(p0[:], s0[:], EXP, bias=zexp[:], scale=SCALE)
                p1 = PTP.tile([128, QW], BF16, tag="pt", name=f"p1_{bi}_{k}")
                nc.scalar.activation(p1[:], s1[:], EXP, bias=zexp[:], scale=SCALE)
                tree_push(0, p0)
                tree_push(1, p1)
                # V projection + leftover pair-0 chunks live in block 0,
                # after the scores so the first exps are not delayed
                if bi == 0:
                    emit_vproj(k)
                    if k in proj0_rest:
                        emit_proj_chunk(proj0, proj0_rest[k])
                # next pair's K/Q projection, split so that everything the
                # NEXT block's k=0 scores read (K0, Q0, Q1; K1 by iter 4)
                # lands by the end of this block; the late K/Q chunks (K2,
                # K3 first read at iters 8/12, Q2/Q3 only by the pair's
                # second q-pass) follow at the START of the next block
                if qp == 1 and lj + 1 < NPAIR:
                    if k == 0:
                        proj_next = emit_proj_dmas(lj + 1)
                        kt_next, qt_next = proj_next[3], proj_next[4]
                    if k % 4 in (1, 2):  # halves at k = 1+2, 5+6, 9+10, 13+14
                        emit_proj_half(
                            proj_next, (0, 4, 5, 1)[(k - 1) // 4], (k - 1) % 4
                        )
                if qp == 0 and lj >= 1 and k % 4 in (0, 1):
                    emit_proj_half(proj_cur, (2, 3, 6, 7)[k // 4], k % 4)
                # PV deferred by two k-iterations: each PV matmul then has
                # two full iterations of exp slack
                prev.append((k, p0, p1))
                if len(prev) == 3:
                    emit_pv(*prev.pop(0))
                # output-projection tiles overlap the last block's
                # exp-paced slack (~10us fits six); the rest pipeline in
                # the tail
                if bi == last and k in (2, 4, 6, 8, 10, 12, 14):
                    emit_phase3(k // 2 - 1)
            for pv_args in prev:
                emit_pv(*pv_args)
            # denominators: single matmul per head reduces the 128 partial
            # sums AND broadcasts to 64 partitions (ones[128,64] weights)
            den0 = rsum[0]
            den1 = rsum[1]
            rc = RCP.tile([128, QW], F32, tag="rc", name=f"rc_{lj}_{qp}")
            for qc in range(2):
                psl = slice(qc * 512, (qc + 1) * 512)
                bch = PS.tile(
                    [128, 512], F32, tag="chunk", bufs=2, name=f"bc_{lj}_{qp}_{qc}"
                )
                nc.tensor.matmul(
                    bch[0:64, :], lhsT=ones_red[:], rhs=den0[:, psl],
                    start=True, stop=True,
                )
                nc.tensor.matmul(
                    bch[64:128, :], lhsT=ones_red[:], rhs=den1[:, psl],
                    start=True, stop=True,
                )
                nc.vector.reciprocal_approx_fast(out=rc[:, psl], in_=bch[:])
            nc.vector.tensor_mul(ctxu[lj][:, q0 : q0 + QW], ctx01[:], rc[:])
            nc.vector.tensor_scalar_add(
                ctxu[lj][:, q0 : q0 + QW],
                ctxu[lj][:, q0 : q0 + QW],
                bv_all[:, lj : lj + 1],
            )
            if qp == NQP - 1 and lj + 1 < NPAIR:
                kt_cur, qt_cur = kt_next, qt_next
                proj_cur = proj_next

        # ---- tail: the remaining output projection ----
        for st in range(7, NT):
            emit_phase3(st, tail=True)


def build_nc():
    nc = bacc.Bacc(
        "TRN2", target_bir_lowering=False, debug=False, num_devices=N_CORES
    )
    tin = {
        "ztc": nc.dram_tensor("ztc", [D, S], BF16, kind="ExternalInput").ap(),
        "wqTc": nc.dram_tensor("wqTc", [D, DH], BF16, kind="ExternalInput").ap(),
        "wkTc": nc.dram_tensor("wkTc", [D, DH], BF16, kind="ExternalInput").ap(),
        "wvTc": nc.dram_tensor("wvTc", [D, DH], BF16, kind="ExternalInput").ap(),
        "woTc": nc.dram_tensor("woTc", [DH, D], BF16, kind="ExternalInput").ap(),
        "bqc": nc.dram_tensor("bqc", [128, NPAIR], F32, kind="ExternalInput").ap(),
        "bvc": nc.dram_tensor("bvc", [128, NPAIR], F32, kind="ExternalInput").ap(),
    }
    tout = {"o": nc.dram_tensor("o", [S, D], F32, kind="ExternalOutput").ap()}
    with tile.TileContext(nc) as tc:
        _emit(tc, tin, tout)
    nc.compile()
    return nc


_NC = None


def _get_nc():
    global _NC
    if _NC is None:
        _NC = build_nc()
    return _NC


def make_in_maps(z, Wq, bq, Wk, Wv, bv, Wo, bo):
    """Build the 8 per-core input maps from full fp32 inputs."""
    z = np.asarray(z, np.float32)
    bq = np.asarray(bq, np.float32)
    bv = np.asarray(bv, np.float32)
    bo = np.asarray(bo, np.float32)
    wqT = np.asarray(Wq, np.float32).T
    wkT = np.asarray(Wk, np.float32).T
    wvT = np.asarray(Wv, np.float32).T
    woT = np.asarray(Wo, np.float32).T
    zts = [np.ascontiguousarray(z[b].T).astype(NPBF16) for b in range(B)]
    per_hg = []
    for hg in range(2):
        dsl = slice(hg * DH, (hg + 1) * DH)
        per_hg.append(
            {
                "wqTc": np.ascontiguousarray(wqT[:, dsl]).astype(NPBF16),
                "wkTc": np.ascontiguousarray(wkT[:, dsl]).astype(NPBF16),
                "wvTc": np.ascontiguousarray(wvT[:, dsl]).astype(NPBF16),
                "woTc": np.ascontiguousarray(woT[dsl, :]).astype(NPBF16),
                "bqc": np.ascontiguousarray(bq[dsl].reshape(NPAIR, 128).T),
                "bvc": np.ascontiguousarray(bv[dsl].reshape(NPAIR, 128).T),
            }
        )
    in_maps = []
    for c in range(N_CORES):
        b, hg = c // 2, c % 2
        in_maps.append({"ztc": zts[b], **per_hg[hg]})
    return in_maps


def run(in_maps, trace=False):
    nc = _get_nc()
    return run_bass_kernel_spmd(
        nc, in_maps, core_ids=list(range(N_CORES)), trace=trace
    )


def kernel(z, Wq, bq, Wk, bk, Wv, bv, Wo, bo):
    in_maps = make_in_maps(z, Wq, bq, Wk, Wv, bv, Wo, bo)
    res = run(in_maps)
    bo32 = np.asarray(bo, np.float32).reshape(1, D)
    out = np.empty((B, S, D), np.float32)
    for b in range(B):
        out[b] = (
            res.results[2 * b]["o"].astype(np.float32)
            + res.results[2 * b + 1]["o"].astype(np.float32)
            + bo32
        )
    return out
